# revision 8
# baseline (speedup 1.0000x reference)
"""Conv3d (8,32,48,48,48) * (64,32,3,3,3) -> (8,64,46,46,46), valid, stride 1.

Data-parallel over batch: 1 image per NeuronCore, 8 cores. Per core the conv
is an implicit GEMM with a combined weight matrix so ONE matmul computes TWO
output planes:
  - the window tile holds 4 input planes d0..d0+3 stacked on 128 partitions
    as (kd, ci),
  - lhsT [128, 128]: cols 0-63 = plane d0's weights on rows 0-95 (kd 0-2),
    cols 64-127 = plane d0+1's weights on rows 32-127 (same values shifted
    one 32-row block down), zeros elsewhere,
  - kh, kw are free-dim offsets into the window tile, so each (kh,kw) tap is
    one accumulating K=128 x M=128 matmul; 9 taps complete a PSUM chunk,
  - tap-outer loop: each tap's weights stay loaded across all 5 row chunks
    (9 LDWEIGHTS per plane pair instead of 90),
  - PSUM -> SBUF with fused bias: chunks 0-2 on ScalarE, 3-4 on VectorE,
  - ONE input DMA and ONE output DMA per plane pair (output rows d*64+co are
    contiguous in HBM), keeping the HWDGE trigger queue nearly idle,
  - warmup matmuls on a scratch PSUM bank release the HAM clock gate while
    the first DMAs are in flight.
"""

import functools

import numpy as np

import concourse.bacc as bacc
import concourse.tile as tile
from concourse import mybir
from concourse.bass_utils import run_bass_kernel_spmd

# Problem constants (hardcoded per harness contract)
B = 8
CI = 32
DIN = 48
CO = 64
K = 3
DOUT = DIN - K + 1  # 46
SPP = DOUT * DOUT  # 2116 spatial positions per output plane
PLANE = DIN * DIN  # 2304 elements per (ci, plane)
NTAP = K * K  # 9 (kh, kw) taps per output chunk

# h'-row chunking of a 46x46 output plane into PSUM-bank-sized matmuls
CHUNKS = [(0, 10), (10, 9), (19, 9), (28, 9), (37, 9)]  # (h0, rows) -> N = rows*46

F32 = mybir.dt.float32
F16 = mybir.dt.float16

WARMUP = 14  # scratch matmuls issued before the first real work


@functools.lru_cache(maxsize=1)
def build_program():
    nc = bacc.Bacc("TRN2", target_bir_lowering=False, debug=False)

    x = nc.dram_tensor("x", [DIN * CI, PLANE], F16, kind="ExternalInput").ap()
    wt = nc.dram_tensor("wt", [128, NTAP * 128], F16, kind="ExternalInput").ap()
    b2 = nc.dram_tensor("b2", [2 * CO, 1], F32, kind="ExternalInput").ap()
    # output rows are d*64 + co so each plane pair is one contiguous DMA
    y = nc.dram_tensor("y", [DOUT * CO, SPP], F32, kind="ExternalOutput").ap()

    with tile.TileContext(nc) as tc:
        with (
            tc.tile_pool(name="wp", bufs=1) as wpool,
            tc.tile_pool(name="xp", bufs=3) as xpool,
            tc.tile_pool(name="op", bufs=3) as opool,
            tc.tile_pool(name="p0", bufs=2, space="PSUM") as p0pool,
            tc.tile_pool(name="pa", bufs=1, space="PSUM") as papool,
            tc.tile_pool(name="ps", bufs=1, space="PSUM") as pspool,
        ):
            # Scratch PSUM bank: warmup target + "wait absorber" dummy matmuls
            # (absorbs DMA-completion waits so real matmuls only wait on their
            # PSUM slot).
            scr = pspool.tile([128, 512], F32)

            # First window tile's DMA goes out before the weights so the
            # critical-path input lands as early as possible.
            xw0 = xpool.tile([128, PLANE], F16, tag="xw", name="xw0")
            nc.sync.dma_start(xw0[:, :], x[0:128, :])

            # Warm up the PE so the HAM clock gate is released (needs ~3.4us
            # of sustained activity) while the first DMAs land.
            wu = wpool.tile([128, 256], F16)
            nc.gpsimd.memset(wu[:, :], 0.0)
            for _ in range(WARMUP):
                nc.tensor.matmul(
                    scr[:, :256], wu[:, :128], wu[:, :256], start=True, stop=True
                )

            wa = wpool.tile([128, NTAP * 128], F16)
            nc.sync.dma_start(wa[:, :], wt)
            bias_t = wpool.tile([2 * CO, 1], F32)
            nc.sync.dma_start(bias_t[:, :], b2)
            # absorb the weights-DMA wait
            nc.tensor.matmul(scr[:, :2], wa[:, :128], wa[:, :2], start=True, stop=True)

            for t in range(DOUT // 2):
                d0 = 2 * t
                # window tile: planes d0..d0+3 stacked on partitions (kd,ci)
                if t == 0:
                    xw = xw0
                else:
                    xw = xpool.tile([128, PLANE], F16, tag="xw", name=f"xw{t}")
                    nc.sync.dma_start(xw[:, :], x[CI * d0 : CI * d0 + 128, :])
                xw3 = xw[:, :].rearrange("p (h w) -> p h w", w=DIN)

                # absorb the window-DMA wait
                nc.tensor.matmul(
                    scr[:, :2], wa[:, :128], xw[:, :2], start=True, stop=True
                )

                pt = [
                    (p0pool if c < 2 else papool).tile(
                        [128, 512], F32, tag=f"pc{c}", name=f"pc{c}_{t}"
                    )
                    for c in range(5)
                ]
                ot = opool.tile([128, SPP], F32)

                for tap in range(NTAP):
                    kh, kw = divmod(tap, K)
                    lhs = wa[:, tap * 128 : (tap + 1) * 128]
                    for c, (h0, rows) in enumerate(CHUNKS):
                        n = rows * DOUT
                        nc.tensor.matmul(
                            pt[c][:, :n],
                            lhs,
                            xw3[:, h0 + kh : h0 + kh + rows, kw : kw + DOUT],
                            start=(tap == 0),
                            stop=(tap == NTAP - 1),
                        )

                last = t == DOUT // 2 - 1
                for c, (h0, rows) in enumerate(CHUNKS):
                    n = rows * DOUT
                    cs = slice(h0 * DOUT, h0 * DOUT + n)
                    if c < 3:
                        nc.scalar.activation(
                            ot[:, cs],
                            pt[c][:, :n],
                            mybir.ActivationFunctionType.Identity,
                            bias=bias_t[:, :],
                        )
                    else:
                        nc.vector.tensor_scalar_add(
                            ot[:, cs], pt[c][:, :n], bias_t[:, :]
                        )
                    if last:
                        # final iteration: store per chunk so the last DMA is
                        # small and the kernel tail is short
                        nc.sync.dma_start(
                            y[CO * d0 : CO * d0 + 2 * CO, cs], ot[:, cs]
                        )

                if not last:
                    nc.sync.dma_start(y[CO * d0 : CO * d0 + 2 * CO, :], ot[:, :])

    nc.compile()
    return nc


def _f16(a):
    return np.ascontiguousarray(np.asarray(a, dtype=np.float32).astype(np.float16))


def make_in_maps(inputs, weight, bias):
    """Host-side shard/pack: returns per-core input maps."""
    inputs = np.ascontiguousarray(np.asarray(inputs, dtype=np.float32))
    weight = np.asarray(weight, dtype=np.float32)
    bias = np.asarray(bias, dtype=np.float32)

    # combined weights: [p=(kd,ci), tap*128 + (plane, co)]
    wt5 = weight.transpose(2, 1, 3, 4, 0)  # [kd, ci, kh, kw, co]
    wcomb = np.zeros((128, NTAP, 128), np.float32)
    for kh in range(K):
        for kw in range(K):
            tap = kh * K + kw
            blk = wt5[:, :, kh, kw, :].reshape(K * CI, CO)  # [(kd ci), co]
            wcomb[0:96, tap, 0:64] = blk  # plane d0: kd 0-2 on rows 0-95
            wcomb[32:128, tap, 64:128] = blk  # plane d1: kd 0-2 on rows 32-127
    wtp = _f16(wcomb.reshape(128, NTAP * 128))

    b2 = np.ascontiguousarray(np.tile(bias, 2).reshape(2 * CO, 1))
    in_maps = []
    for c in range(B):
        xc = _f16(inputs[c].transpose(1, 0, 2, 3).reshape(DIN * CI, PLANE))
        in_maps.append({"x": xc, "wt": wtp, "b2": b2})
    return in_maps


def kernel(inputs, weight, bias, **run_kwargs):
    nc = build_program()
    in_maps = make_in_maps(inputs, weight, bias)
    res = run_bass_kernel_spmd(nc, in_maps, core_ids=list(range(B)), **run_kwargs)
    out = np.stack(
        [
            res.results[c]["y"]
            .reshape(DOUT, CO, SPP)
            .transpose(1, 0, 2)
            .reshape(CO, DOUT, DOUT, DOUT)
            for c in range(B)
        ]
    )
    return out.astype(np.float32)


# revision 12
# speedup vs baseline: 1.0085x; 1.0085x over previous
"""Conv3d (8,32,48,48,48) * (64,32,3,3,3) -> (8,64,46,46,46), valid, stride 1.

Data-parallel over batch: 1 image per NeuronCore, 8 cores. Per core the conv
is an implicit GEMM with a combined weight matrix so ONE matmul computes TWO
output planes:
  - the window tile holds 4 input planes d0..d0+3 stacked on 128 partitions
    as (kd, ci),
  - lhsT [128, 128]: cols 0-63 = plane d0's weights on rows 0-95 (kd 0-2),
    cols 64-127 = plane d0+1's weights on rows 32-127 (same values shifted
    one 32-row block down), zeros elsewhere,
  - kh, kw are free-dim offsets into the window tile, so each (kh,kw) tap is
    one accumulating K=128 x M=128 matmul; 9 taps complete a PSUM chunk,
  - tap-outer loop: each tap's weights stay loaded across all 5 row chunks
    (9 LDWEIGHTS per plane pair instead of 90),
  - PSUM -> SBUF with fused bias: chunks 0-2 on ScalarE, 3-4 on VectorE,
  - ONE input DMA and ONE output DMA per plane pair (output rows d*64+co are
    contiguous in HBM), keeping the HWDGE trigger queue nearly idle,
  - warmup matmuls on a scratch PSUM bank release the HAM clock gate while
    the first DMAs are in flight.
"""

import functools

import numpy as np

import concourse.bacc as bacc
import concourse.tile as tile
from concourse import mybir
from concourse.bass_utils import run_bass_kernel_spmd

# Problem constants (hardcoded per harness contract)
B = 8
CI = 32
DIN = 48
CO = 64
K = 3
DOUT = DIN - K + 1  # 46
SPP = DOUT * DOUT  # 2116 spatial positions per output plane
PLANE = DIN * DIN  # 2304 elements per (ci, plane)
NTAP = K * K  # 9 (kh, kw) taps per output chunk

# h'-row chunking of a 46x46 output plane into PSUM-bank-sized matmuls
CHUNKS = [(0, 10), (10, 9), (19, 9), (28, 9), (37, 9)]  # (h0, rows) -> N = rows*46

F32 = mybir.dt.float32
F16 = mybir.dt.float16

WARMUP = 11  # scratch matmuls issued before the first real work

# row bands of the first window tile, DMA'd separately so chunk c's matmuls
# can start as soon as band c lands: band c covers rows h0..h0+rows+2
BANDS = [(0, 12), (10, 12), (19, 12), (28, 12), (37, 11)]  # (row0, nrows)


@functools.lru_cache(maxsize=1)
def build_program():
    nc = bacc.Bacc("TRN2", target_bir_lowering=False, debug=False)

    x = nc.dram_tensor("x", [DIN * CI, PLANE], F16, kind="ExternalInput").ap()
    wt = nc.dram_tensor("wt", [128, NTAP * 128], F16, kind="ExternalInput").ap()
    b2 = nc.dram_tensor("b2", [2 * CO, 1], F32, kind="ExternalInput").ap()
    # output rows are d*64 + co so each plane pair is one contiguous DMA
    y = nc.dram_tensor("y", [DOUT * CO, SPP], F32, kind="ExternalOutput").ap()

    with tile.TileContext(nc) as tc:
        with (
            tc.tile_pool(name="wp", bufs=1) as wpool,
            tc.tile_pool(name="xp", bufs=3) as xpool,
            tc.tile_pool(name="op", bufs=3) as opool,
            tc.tile_pool(name="p0", bufs=2, space="PSUM") as p0pool,
            tc.tile_pool(name="pa", bufs=1, space="PSUM") as papool,
            tc.tile_pool(name="ps", bufs=1, space="PSUM") as pspool,
        ):
            # Scratch PSUM bank: warmup target + "wait absorber" dummy matmuls
            # (absorbs DMA-completion waits so real matmuls only wait on their
            # PSUM slot).
            scr = pspool.tile([128, 512], F32)

            # Weights + bias ride the ACT HWDGE ring, the first window tile's
            # row bands ride the SP ring: the two transfer in parallel and the
            # first chunk's matmuls only wait for band 0 (~130 KB).
            wa = wpool.tile([128, NTAP * 128], F16)
            nc.scalar.dma_start(wa[:, :], wt)
            bias_t = wpool.tile([2 * CO, 1], F32)
            nc.scalar.dma_start(bias_t[:, :], b2)

            xw0 = xpool.tile([128, PLANE], F16, tag="xw", name="xw0")
            for r0, nr in BANDS:
                nc.sync.dma_start(
                    xw0[:, r0 * DIN : (r0 + nr) * DIN],
                    x[0:128, r0 * DIN : (r0 + nr) * DIN],
                )

            # Warm up the PE so the HAM clock gate is released (needs ~3.4us
            # of sustained activity) while the first DMAs land.
            wu = wpool.tile([128, 256], F16)
            nc.gpsimd.memset(wu[:, :], 0.0)
            for _ in range(WARMUP):
                nc.tensor.matmul(
                    scr[:, :256], wu[:, :128], wu[:, :256], start=True, stop=True
                )

            # absorb the weights-DMA wait
            nc.tensor.matmul(scr[:, :2], wa[:, :128], wa[:, :2], start=True, stop=True)

            for t in range(DOUT // 2):
                d0 = 2 * t
                # window tile: planes d0..d0+3 stacked on partitions (kd,ci)
                if t == 0:
                    xw = xw0
                else:
                    xw = xpool.tile([128, PLANE], F16, tag="xw", name=f"xw{t}")
                    nc.sync.dma_start(xw[:, :], x[CI * d0 : CI * d0 + 128, :])
                    # absorb the window-DMA wait
                    nc.tensor.matmul(
                        scr[:, :2], wa[:, :128], xw[:, :2], start=True, stop=True
                    )
                xw3 = xw[:, :].rearrange("p (h w) -> p h w", w=DIN)

                pt = [
                    (p0pool if c < 2 else papool).tile(
                        [128, 512], F32, tag=f"pc{c}", name=f"pc{c}_{t}"
                    )
                    for c in range(5)
                ]
                ot = opool.tile([128, SPP], F32)

                if t == 0:
                    # chunk-outer on the first iteration: chunk c only needs
                    # band c of the window tile, so matmuls start on band 0
                    for c, (h0, rows) in enumerate(CHUNKS):
                        n = rows * DOUT
                        for tap in range(NTAP):
                            kh, kw = divmod(tap, K)
                            nc.tensor.matmul(
                                pt[c][:, :n],
                                wa[:, tap * 128 : (tap + 1) * 128],
                                xw3[:, h0 + kh : h0 + kh + rows, kw : kw + DOUT],
                                start=(tap == 0),
                                stop=(tap == NTAP - 1),
                            )
                else:
                    for tap in range(NTAP):
                        kh, kw = divmod(tap, K)
                        lhs = wa[:, tap * 128 : (tap + 1) * 128]
                        for c, (h0, rows) in enumerate(CHUNKS):
                            n = rows * DOUT
                            nc.tensor.matmul(
                                pt[c][:, :n],
                                lhs,
                                xw3[:, h0 + kh : h0 + kh + rows, kw : kw + DOUT],
                                start=(tap == 0),
                                stop=(tap == NTAP - 1),
                            )

                last = t == DOUT // 2 - 1
                for c, (h0, rows) in enumerate(CHUNKS):
                    n = rows * DOUT
                    cs = slice(h0 * DOUT, h0 * DOUT + n)
                    if c < 3:
                        nc.scalar.activation(
                            ot[:, cs],
                            pt[c][:, :n],
                            mybir.ActivationFunctionType.Identity,
                            bias=bias_t[:, :],
                        )
                    else:
                        nc.vector.tensor_scalar_add(
                            ot[:, cs], pt[c][:, :n], bias_t[:, :]
                        )
                    if last:
                        # final iteration: store per chunk so the last DMA is
                        # small and the kernel tail is short
                        nc.scalar.dma_start(
                            y[CO * d0 : CO * d0 + 2 * CO, cs], ot[:, cs]
                        )

                if not last:
                    nc.scalar.dma_start(y[CO * d0 : CO * d0 + 2 * CO, :], ot[:, :])

    nc.compile()
    return nc


def _f16(a):
    return np.ascontiguousarray(np.asarray(a, dtype=np.float32).astype(np.float16))


def make_in_maps(inputs, weight, bias):
    """Host-side shard/pack: returns per-core input maps."""
    inputs = np.ascontiguousarray(np.asarray(inputs, dtype=np.float32))
    weight = np.asarray(weight, dtype=np.float32)
    bias = np.asarray(bias, dtype=np.float32)

    # combined weights: [p=(kd,ci), tap*128 + (plane, co)]
    wt5 = weight.transpose(2, 1, 3, 4, 0)  # [kd, ci, kh, kw, co]
    wcomb = np.zeros((128, NTAP, 128), np.float32)
    for kh in range(K):
        for kw in range(K):
            tap = kh * K + kw
            blk = wt5[:, :, kh, kw, :].reshape(K * CI, CO)  # [(kd ci), co]
            wcomb[0:96, tap, 0:64] = blk  # plane d0: kd 0-2 on rows 0-95
            wcomb[32:128, tap, 64:128] = blk  # plane d1: kd 0-2 on rows 32-127
    wtp = _f16(wcomb.reshape(128, NTAP * 128))

    b2 = np.ascontiguousarray(np.tile(bias, 2).reshape(2 * CO, 1))
    in_maps = []
    for c in range(B):
        xc = _f16(inputs[c].transpose(1, 0, 2, 3).reshape(DIN * CI, PLANE))
        in_maps.append({"x": xc, "wt": wtp, "b2": b2})
    return in_maps


def kernel(inputs, weight, bias, **run_kwargs):
    nc = build_program()
    in_maps = make_in_maps(inputs, weight, bias)
    res = run_bass_kernel_spmd(nc, in_maps, core_ids=list(range(B)), **run_kwargs)
    out = np.stack(
        [
            res.results[c]["y"]
            .reshape(DOUT, CO, SPP)
            .transpose(1, 0, 2)
            .reshape(CO, DOUT, DOUT, DOUT)
            for c in range(B)
        ]
    )
    return out.astype(np.float32)


# revision 17
# speedup vs baseline: 1.0109x; 1.0025x over previous
"""Conv3d (8,32,48,48,48) * (64,32,3,3,3) -> (8,64,46,46,46), valid, stride 1.

Data-parallel over batch: 1 image per NeuronCore, 8 cores. Per core the conv
is an implicit GEMM with a combined weight matrix so ONE matmul computes TWO
output planes:
  - the window tile holds 4 input planes d0..d0+3 stacked on 128 partitions
    as (kd, ci),
  - lhsT [128, 128]: cols 0-63 = plane d0's weights on rows 0-95 (kd 0-2),
    cols 64-127 = plane d0+1's weights on rows 32-127 (same values shifted
    one 32-row block down), zeros elsewhere,
  - kh, kw are free-dim offsets into the window tile, so each (kh,kw) tap is
    one accumulating K=128 x M=128 matmul; 9 taps complete a PSUM chunk,
  - tap-outer loop: each tap's weights stay loaded across all 5 row chunks
    (9 LDWEIGHTS per plane pair instead of 90),
  - PSUM -> SBUF with fused bias: chunks 0-2 on ScalarE, 3-4 on VectorE,
  - ONE input DMA and ONE output DMA per plane pair (output rows d*64+co are
    contiguous in HBM), keeping the HWDGE trigger queue nearly idle,
  - warmup matmuls on a scratch PSUM bank release the HAM clock gate while
    the first DMAs are in flight.
"""

import functools

import numpy as np

import concourse.bacc as bacc
import concourse.tile as tile
from concourse import mybir
from concourse.bass_utils import run_bass_kernel_spmd

# Problem constants (hardcoded per harness contract)
B = 8
CI = 32
DIN = 48
CO = 64
K = 3
DOUT = DIN - K + 1  # 46
SPP = DOUT * DOUT  # 2116 spatial positions per output plane
PLANE = DIN * DIN  # 2304 elements per (ci, plane)
NTAP = K * K  # 9 (kh, kw) taps per output chunk

# h'-row chunking of a 46x46 output plane into PSUM-bank-sized matmuls
CHUNKS = [(0, 10), (10, 9), (19, 9), (28, 9), (37, 9)]  # (h0, rows) -> N = rows*46

F32 = mybir.dt.float32
F16 = mybir.dt.float16

WARMUP = 11  # scratch matmuls issued before the first real work

# row bands of the first window tile, DMA'd separately so chunk c's matmuls
# can start as soon as band c lands: band c covers rows h0..h0+rows+2
BANDS = [(0, 12), (10, 12), (19, 12), (28, 12), (37, 11)]  # (row0, nrows)


@functools.lru_cache(maxsize=1)
def build_program():
    nc = bacc.Bacc("TRN2", target_bir_lowering=False, debug=False)

    x = nc.dram_tensor("x", [DIN * CI, PLANE], F16, kind="ExternalInput").ap()
    wt = nc.dram_tensor("wt", [128, NTAP * 128], F16, kind="ExternalInput").ap()
    b2 = nc.dram_tensor("b2", [2 * CO, 1], F32, kind="ExternalInput").ap()
    # output rows are d*64 + co so each plane pair is one contiguous DMA
    y = nc.dram_tensor("y", [DOUT * CO, SPP], F32, kind="ExternalOutput").ap()

    with tile.TileContext(nc) as tc:
        with (
            tc.tile_pool(name="wp", bufs=1) as wpool,
            tc.tile_pool(name="xp", bufs=3) as xpool,
            tc.tile_pool(name="op", bufs=3) as opool,
            tc.tile_pool(name="p0", bufs=2, space="PSUM") as p0pool,
            tc.tile_pool(name="pa", bufs=1, space="PSUM") as papool,
            tc.tile_pool(name="ps", bufs=1, space="PSUM") as pspool,
        ):
            # Scratch PSUM bank: warmup target + "wait absorber" dummy matmuls
            # (absorbs DMA-completion waits so real matmuls only wait on their
            # PSUM slot).
            scr = pspool.tile([128, 512], F32)

            # Weights + bias ride the ACT HWDGE ring, the first window tile's
            # row bands ride the SP ring: the two transfer in parallel and the
            # first chunk's matmuls only wait for band 0 (~150 KB) plus the
            # first three taps' weights (~100 KB). Each band is its own tile
            # so the DMAs carry no write-after-write deps and flow in parallel.
            wa_a = wpool.tile([128, 3 * 128], F16)
            nc.scalar.dma_start(wa_a[:, :], wt[:, : 3 * 128])
            wa_b = wpool.tile([128, 6 * 128], F16)
            nc.scalar.dma_start(wa_b[:, :], wt[:, 3 * 128 :])
            bias_t = wpool.tile([2 * CO, 1], F32)
            nc.scalar.dma_start(bias_t[:, :], b2)

            bt = []
            for c, (r0, nr) in enumerate(BANDS):
                btile = wpool.tile([128, nr * DIN], F16, name=f"band{c}")
                nc.sync.dma_start(
                    btile[:, :], x[0:128, r0 * DIN : (r0 + nr) * DIN]
                )
                bt.append(btile)

            # Warm up the PE so the HAM clock gate is released (needs ~3.4us
            # of sustained activity) while the first DMAs land.
            wu = wpool.tile([128, 256], F16)
            nc.gpsimd.memset(wu[:, :], 0.0)
            for _ in range(WARMUP):
                nc.tensor.matmul(
                    scr[:, :256], wu[:, :128], wu[:, :256], start=True, stop=True
                )

            # absorb the weights-DMA wait (wa_a gates the first real matmul)
            nc.tensor.matmul(
                scr[:, :2], wa_a[:, :128], wa_a[:, :2], start=True, stop=True
            )

            def wtap(tap):
                if tap < 3:
                    return wa_a[:, tap * 128 : (tap + 1) * 128]
                return wa_b[:, (tap - 3) * 128 : (tap - 2) * 128]


            for t in range(DOUT // 2):
                d0 = 2 * t
                # window tile: planes d0..d0+3 stacked on partitions (kd,ci)
                if t > 0:
                    xw = xpool.tile([128, PLANE], F16, tag="xw", name=f"xw{t}")
                    nc.sync.dma_start(xw[:, :], x[CI * d0 : CI * d0 + 128, :])
                    # absorb the window-DMA wait
                    nc.tensor.matmul(
                        scr[:, :2], wa_a[:, :128], xw[:, :2], start=True, stop=True
                    )
                    xw3 = xw[:, :].rearrange("p (h w) -> p h w", w=DIN)

                pt = [
                    (p0pool if c < 2 else papool).tile(
                        [128, 512], F32, tag=f"pc{c}", name=f"pc{c}_{t}"
                    )
                    for c in range(5)
                ]
                ot = opool.tile([128, SPP], F32)

                if t == 0:
                    # chunk-outer on the first iteration: chunk c only needs
                    # band tile c, so matmuls start as soon as band 0 lands
                    for c, (h0, rows) in enumerate(CHUNKS):
                        n = rows * DOUT
                        bt3 = bt[c][:, :].rearrange("p (h w) -> p h w", w=DIN)
                        for tap in range(NTAP):
                            kh, kw = divmod(tap, K)
                            nc.tensor.matmul(
                                pt[c][:, :n],
                                wtap(tap),
                                bt3[:, kh : kh + rows, kw : kw + DOUT],
                                start=(tap == 0),
                                stop=(tap == NTAP - 1),
                            )
                else:
                    for tap in range(NTAP):
                        kh, kw = divmod(tap, K)
                        lhs = wtap(tap)
                        for c, (h0, rows) in enumerate(CHUNKS):
                            n = rows * DOUT
                            nc.tensor.matmul(
                                pt[c][:, :n],
                                lhs,
                                xw3[:, h0 + kh : h0 + kh + rows, kw : kw + DOUT],
                                start=(tap == 0),
                                stop=(tap == NTAP - 1),
                            )

                last = t == DOUT // 2 - 1
                for c, (h0, rows) in enumerate(CHUNKS):
                    n = rows * DOUT
                    cs = slice(h0 * DOUT, h0 * DOUT + n)
                    if c < 3:
                        nc.scalar.activation(
                            ot[:, cs],
                            pt[c][:, :n],
                            mybir.ActivationFunctionType.Identity,
                            bias=bias_t[:, :],
                        )
                    else:
                        nc.vector.tensor_scalar_add(
                            ot[:, cs], pt[c][:, :n], bias_t[:, :]
                        )
                    if last:
                        # final iteration: store per chunk so the last DMA is
                        # small and the kernel tail is short
                        nc.scalar.dma_start(
                            y[CO * d0 : CO * d0 + 2 * CO, cs], ot[:, cs]
                        )

                if not last:
                    nc.scalar.dma_start(y[CO * d0 : CO * d0 + 2 * CO, :], ot[:, :])

    nc.compile()
    return nc


def _f16(a):
    return np.ascontiguousarray(np.asarray(a, dtype=np.float32).astype(np.float16))


def make_in_maps(inputs, weight, bias):
    """Host-side shard/pack: returns per-core input maps."""
    inputs = np.ascontiguousarray(np.asarray(inputs, dtype=np.float32))
    weight = np.asarray(weight, dtype=np.float32)
    bias = np.asarray(bias, dtype=np.float32)

    # combined weights: [p=(kd,ci), tap*128 + (plane, co)]
    wt5 = weight.transpose(2, 1, 3, 4, 0)  # [kd, ci, kh, kw, co]
    wcomb = np.zeros((128, NTAP, 128), np.float32)
    for kh in range(K):
        for kw in range(K):
            tap = kh * K + kw
            blk = wt5[:, :, kh, kw, :].reshape(K * CI, CO)  # [(kd ci), co]
            wcomb[0:96, tap, 0:64] = blk  # plane d0: kd 0-2 on rows 0-95
            wcomb[32:128, tap, 64:128] = blk  # plane d1: kd 0-2 on rows 32-127
    wtp = _f16(wcomb.reshape(128, NTAP * 128))

    b2 = np.ascontiguousarray(np.tile(bias, 2).reshape(2 * CO, 1))
    in_maps = []
    for c in range(B):
        xc = _f16(inputs[c].transpose(1, 0, 2, 3).reshape(DIN * CI, PLANE))
        in_maps.append({"x": xc, "wt": wtp, "b2": b2})
    return in_maps


def kernel(inputs, weight, bias, **run_kwargs):
    nc = build_program()
    in_maps = make_in_maps(inputs, weight, bias)
    res = run_bass_kernel_spmd(nc, in_maps, core_ids=list(range(B)), **run_kwargs)
    out = np.stack(
        [
            res.results[c]["y"]
            .reshape(DOUT, CO, SPP)
            .transpose(1, 0, 2)
            .reshape(CO, DOUT, DOUT, DOUT)
            for c in range(B)
        ]
    )
    return out.astype(np.float32)


# revision 18
# speedup vs baseline: 1.0198x; 1.0088x over previous
"""Conv3d (8,32,48,48,48) * (64,32,3,3,3) -> (8,64,46,46,46), valid, stride 1.

Data-parallel over batch: 1 image per NeuronCore, 8 cores. Per core the conv
is an implicit GEMM with a combined weight matrix so ONE matmul computes TWO
output planes:
  - the window tile holds 4 input planes d0..d0+3 stacked on 128 partitions
    as (kd, ci),
  - lhsT [128, 128]: cols 0-63 = plane d0's weights on rows 0-95 (kd 0-2),
    cols 64-127 = plane d0+1's weights on rows 32-127 (same values shifted
    one 32-row block down), zeros elsewhere,
  - kh, kw are free-dim offsets into the window tile, so each (kh,kw) tap is
    one accumulating K=128 x M=128 matmul; 9 taps complete a PSUM chunk,
  - tap-outer loop: each tap's weights stay loaded across all 5 row chunks
    (9 LDWEIGHTS per plane pair instead of 90),
  - PSUM -> SBUF with fused bias: chunks 0-2 on ScalarE, 3-4 on VectorE,
  - ONE input DMA and ONE output DMA per plane pair (output rows d*64+co are
    contiguous in HBM), keeping the HWDGE trigger queue nearly idle,
  - warmup matmuls on a scratch PSUM bank release the HAM clock gate while
    the first DMAs are in flight.
"""

import functools

import numpy as np

import concourse.bacc as bacc
import concourse.tile as tile
from concourse import mybir
from concourse.bass_utils import run_bass_kernel_spmd

# Problem constants (hardcoded per harness contract)
B = 8
CI = 32
DIN = 48
CO = 64
K = 3
DOUT = DIN - K + 1  # 46
SPP = DOUT * DOUT  # 2116 spatial positions per output plane
PLANE = DIN * DIN  # 2304 elements per (ci, plane)
NTAP = K * K  # 9 (kh, kw) taps per output chunk

# h'-row chunking of a 46x46 output plane into PSUM-bank-sized matmuls
CHUNKS = [(0, 10), (10, 9), (19, 9), (28, 9), (37, 9)]  # (h0, rows) -> N = rows*46

F32 = mybir.dt.float32
F16 = mybir.dt.float16

WARMUP = 32  # scratch matmuls that keep the PE busy until the first DMAs land


@functools.lru_cache(maxsize=1)
def build_program():
    nc = bacc.Bacc("TRN2", target_bir_lowering=False, debug=False)

    x = nc.dram_tensor("x", [DIN * CI, PLANE], F16, kind="ExternalInput").ap()
    wt = nc.dram_tensor("wt", [128, NTAP * 128], F16, kind="ExternalInput").ap()
    b2 = nc.dram_tensor("b2", [2 * CO, 1], F32, kind="ExternalInput").ap()
    # output rows are d*64 + co so each plane pair is one contiguous DMA
    y = nc.dram_tensor("y", [DOUT * CO, SPP], F32, kind="ExternalOutput").ap()

    with tile.TileContext(nc) as tc:
        with (
            tc.tile_pool(name="wp", bufs=1) as wpool,
            tc.tile_pool(name="xp", bufs=3) as xpool,
            tc.tile_pool(name="op", bufs=3) as opool,
            tc.tile_pool(name="p0", bufs=2, space="PSUM") as p0pool,
            tc.tile_pool(name="pa", bufs=1, space="PSUM") as papool,
            tc.tile_pool(name="ps", bufs=1, space="PSUM") as pspool,
        ):
            # Scratch PSUM bank: warmup target + "wait absorber" dummy matmuls
            # (absorbs DMA-completion waits so real matmuls only wait on their
            # PSUM slot).
            scr = pspool.tile([128, 512], F32)

            # Weights + bias ride the ACT HWDGE ring, the first window tile
            # rides the SP ring: both transfer in parallel while warmup
            # matmuls hold the PE busy (HAM flips to full clock ~3.4us in, so
            # the real stream starts warm the moment the data lands).
            wa = wpool.tile([128, NTAP * 128], F16)
            nc.scalar.dma_start(wa[:, :], wt)
            bias_t = wpool.tile([2 * CO, 1], F32)
            nc.scalar.dma_start(bias_t[:, :], b2)

            xw0 = xpool.tile([128, PLANE], F16, tag="xw", name="xw0")
            nc.sync.dma_start(xw0[:, :], x[0:128, :])

            wu = wpool.tile([128, 256], F16)
            nc.gpsimd.memset(wu[:, :], 0.0)
            for _ in range(WARMUP):
                nc.tensor.matmul(
                    scr[:, :256], wu[:, :128], wu[:, :256], start=True, stop=True
                )

            # absorb the weights-DMA wait
            nc.tensor.matmul(scr[:, :2], wa[:, :128], wa[:, :2], start=True, stop=True)


            for t in range(DOUT // 2):
                d0 = 2 * t
                # window tile: planes d0..d0+3 stacked on partitions (kd,ci)
                if t == 0:
                    xw = xw0
                else:
                    xw = xpool.tile([128, PLANE], F16, tag="xw", name=f"xw{t}")
                    nc.sync.dma_start(xw[:, :], x[CI * d0 : CI * d0 + 128, :])
                    # absorb the window-DMA wait
                    nc.tensor.matmul(
                        scr[:, :2], wa[:, :128], xw[:, :2], start=True, stop=True
                    )
                xw3 = xw[:, :].rearrange("p (h w) -> p h w", w=DIN)

                pt = [
                    (p0pool if c < 2 else papool).tile(
                        [128, 512], F32, tag=f"pc{c}", name=f"pc{c}_{t}"
                    )
                    for c in range(5)
                ]
                ot = opool.tile([128, SPP], F32)

                for tap in range(NTAP):
                    kh, kw = divmod(tap, K)
                    lhs = wa[:, tap * 128 : (tap + 1) * 128]
                    for c, (h0, rows) in enumerate(CHUNKS):
                        n = rows * DOUT
                        nc.tensor.matmul(
                            pt[c][:, :n],
                            lhs,
                            xw3[:, h0 + kh : h0 + kh + rows, kw : kw + DOUT],
                            start=(tap == 0),
                            stop=(tap == NTAP - 1),
                        )

                last = t == DOUT // 2 - 1
                for c, (h0, rows) in enumerate(CHUNKS):
                    n = rows * DOUT
                    cs = slice(h0 * DOUT, h0 * DOUT + n)
                    if c < 3:
                        nc.scalar.activation(
                            ot[:, cs],
                            pt[c][:, :n],
                            mybir.ActivationFunctionType.Identity,
                            bias=bias_t[:, :],
                        )
                    else:
                        nc.vector.tensor_scalar_add(
                            ot[:, cs], pt[c][:, :n], bias_t[:, :]
                        )
                    if last:
                        # final iteration: store per chunk, alternating HWDGE
                        # rings so the triggers issue in parallel and the
                        # kernel tail is short
                        eng = nc.scalar if c % 2 == 0 else nc.sync
                        eng.dma_start(y[CO * d0 : CO * d0 + 2 * CO, cs], ot[:, cs])

                if not last:
                    nc.scalar.dma_start(y[CO * d0 : CO * d0 + 2 * CO, :], ot[:, :])

    nc.compile()
    return nc


def _f16(a):
    return np.ascontiguousarray(np.asarray(a, dtype=np.float32).astype(np.float16))


def make_in_maps(inputs, weight, bias):
    """Host-side shard/pack: returns per-core input maps."""
    inputs = np.ascontiguousarray(np.asarray(inputs, dtype=np.float32))
    weight = np.asarray(weight, dtype=np.float32)
    bias = np.asarray(bias, dtype=np.float32)

    # combined weights: [p=(kd,ci), tap*128 + (plane, co)]
    wt5 = weight.transpose(2, 1, 3, 4, 0)  # [kd, ci, kh, kw, co]
    wcomb = np.zeros((128, NTAP, 128), np.float32)
    for kh in range(K):
        for kw in range(K):
            tap = kh * K + kw
            blk = wt5[:, :, kh, kw, :].reshape(K * CI, CO)  # [(kd ci), co]
            wcomb[0:96, tap, 0:64] = blk  # plane d0: kd 0-2 on rows 0-95
            wcomb[32:128, tap, 64:128] = blk  # plane d1: kd 0-2 on rows 32-127
    wtp = _f16(wcomb.reshape(128, NTAP * 128))

    b2 = np.ascontiguousarray(np.tile(bias, 2).reshape(2 * CO, 1))
    in_maps = []
    for c in range(B):
        xc = _f16(inputs[c].transpose(1, 0, 2, 3).reshape(DIN * CI, PLANE))
        in_maps.append({"x": xc, "wt": wtp, "b2": b2})
    return in_maps


def kernel(inputs, weight, bias, **run_kwargs):
    nc = build_program()
    in_maps = make_in_maps(inputs, weight, bias)
    res = run_bass_kernel_spmd(nc, in_maps, core_ids=list(range(B)), **run_kwargs)
    out = np.stack(
        [
            res.results[c]["y"]
            .reshape(DOUT, CO, SPP)
            .transpose(1, 0, 2)
            .reshape(CO, DOUT, DOUT, DOUT)
            for c in range(B)
        ]
    )
    return out.astype(np.float32)


# revision 19
# speedup vs baseline: 1.0284x; 1.0084x over previous
"""Conv3d (8,32,48,48,48) * (64,32,3,3,3) -> (8,64,46,46,46), valid, stride 1.

Data-parallel over batch: 1 image per NeuronCore, 8 cores. Per core the conv
is an implicit GEMM with a combined weight matrix so ONE matmul computes TWO
output planes:
  - the window tile holds 4 input planes d0..d0+3 stacked on 128 partitions
    as (kd, ci),
  - lhsT [128, 128]: cols 0-63 = plane d0's weights on rows 0-95 (kd 0-2),
    cols 64-127 = plane d0+1's weights on rows 32-127 (same values shifted
    one 32-row block down), zeros elsewhere,
  - kh, kw are free-dim offsets into the window tile, so each (kh,kw) tap is
    one accumulating K=128 x M=128 matmul; 9 taps complete a PSUM chunk,
  - tap-outer loop: each tap's weights stay loaded across all 5 row chunks
    (9 LDWEIGHTS per plane pair instead of 90),
  - PSUM -> SBUF with fused bias: chunks 0-2 on ScalarE, 3-4 on VectorE,
  - ONE input DMA and ONE output DMA per plane pair (output rows d*64+co are
    contiguous in HBM), keeping the HWDGE trigger queue nearly idle,
  - warmup matmuls on a scratch PSUM bank release the HAM clock gate while
    the first DMAs are in flight.
"""

import functools

import numpy as np

import concourse.bacc as bacc
import concourse.tile as tile
from concourse import mybir
from concourse.bass_utils import run_bass_kernel_spmd

# Problem constants (hardcoded per harness contract)
B = 8
CI = 32
DIN = 48
CO = 64
K = 3
DOUT = DIN - K + 1  # 46
SPP = DOUT * DOUT  # 2116 spatial positions per output plane
PLANE = DIN * DIN  # 2304 elements per (ci, plane)
NTAP = K * K  # 9 (kh, kw) taps per output chunk

# h'-row chunking of a 46x46 output plane into PSUM-bank-sized matmuls
CHUNKS = [(0, 10), (10, 9), (19, 9), (28, 9), (37, 9)]  # (h0, rows) -> N = rows*46

F32 = mybir.dt.float32
F16 = mybir.dt.float16

WARMUP = 32  # scratch matmuls that keep the PE busy until the first DMAs land


@functools.lru_cache(maxsize=1)
def build_program():
    nc = bacc.Bacc("TRN2", target_bir_lowering=False, debug=False)

    x = nc.dram_tensor("x", [DIN * CI, PLANE], F16, kind="ExternalInput").ap()
    wt = nc.dram_tensor("wt", [128, NTAP * 128], F16, kind="ExternalInput").ap()
    b2 = nc.dram_tensor("b2", [2 * CO, 1], F32, kind="ExternalInput").ap()
    # output rows are d*64 + co so each plane pair is one contiguous DMA
    y = nc.dram_tensor("y", [DOUT * CO, SPP], F32, kind="ExternalOutput").ap()

    with tile.TileContext(nc) as tc:
        with (
            tc.tile_pool(name="wp", bufs=1) as wpool,
            tc.tile_pool(name="xp", bufs=3) as xpool,
            tc.tile_pool(name="op", bufs=3) as opool,
            tc.tile_pool(name="p0", bufs=2, space="PSUM") as p0pool,
            tc.tile_pool(name="pa", bufs=1, space="PSUM") as papool,
            tc.tile_pool(name="ps", bufs=1, space="PSUM") as pspool,
        ):
            # Scratch PSUM bank: warmup target + "wait absorber" dummy matmuls
            # (absorbs DMA-completion waits so real matmuls only wait on their
            # PSUM slot).
            scr = pspool.tile([128, 512], F32)

            # Weights + bias ride the ACT HWDGE ring, the first window tile
            # rides the SP ring: both transfer in parallel while warmup
            # matmuls hold the PE busy (HAM flips to full clock ~3.4us in, so
            # the real stream starts warm the moment the data lands).
            wa = wpool.tile([128, NTAP * 128], F16)
            nc.scalar.dma_start(wa[:, :], wt)
            bias_t = wpool.tile([2 * CO, 1], F32)
            nc.scalar.dma_start(bias_t[:, :], b2)

            xw0 = xpool.tile([128, PLANE], F16, tag="xw", name="xw0")
            nc.sync.dma_start(xw0[:, :], x[0:128, :])

            wu = wpool.tile([128, 256], F16)
            nc.gpsimd.memset(wu[:, :], 0.0)
            for _ in range(WARMUP):
                nc.tensor.matmul(
                    scr[:, :256], wu[:, :128], wu[:, :256], start=True, stop=True
                )

            # absorb the weights-DMA wait
            nc.tensor.matmul(scr[:, :2], wa[:, :128], wa[:, :2], start=True, stop=True)


            for t in range(DOUT // 2):
                d0 = 2 * t
                # window tile: planes d0..d0+3 stacked on partitions (kd,ci)
                if t == 0:
                    xw = xw0
                else:
                    xw = xpool.tile([128, PLANE], F16, tag="xw", name=f"xw{t}")
                    nc.sync.dma_start(xw[:, :], x[CI * d0 : CI * d0 + 128, :])
                    # absorb the window-DMA wait
                    nc.tensor.matmul(
                        scr[:, :2], wa[:, :128], xw[:, :2], start=True, stop=True
                    )
                xw3 = xw[:, :].rearrange("p (h w) -> p h w", w=DIN)

                pt = [
                    (p0pool if c < 2 else papool).tile(
                        [128, 512], F32, tag=f"pc{c}", name=f"pc{c}_{t}"
                    )
                    for c in range(5)
                ]
                ot = opool.tile([128, SPP], F32)

                last = t == DOUT // 2 - 1

                def copy_chunk(c, h0, rows):
                    n = rows * DOUT
                    cs = slice(h0 * DOUT, h0 * DOUT + n)
                    if c < 3:
                        nc.scalar.activation(
                            ot[:, cs],
                            pt[c][:, :n],
                            mybir.ActivationFunctionType.Identity,
                            bias=bias_t[:, :],
                        )
                    else:
                        nc.vector.tensor_scalar_add(
                            ot[:, cs], pt[c][:, :n], bias_t[:, :]
                        )
                    return cs

                if not last:
                    for tap in range(NTAP):
                        kh, kw = divmod(tap, K)
                        lhs = wa[:, tap * 128 : (tap + 1) * 128]
                        for c, (h0, rows) in enumerate(CHUNKS):
                            n = rows * DOUT
                            nc.tensor.matmul(
                                pt[c][:, :n],
                                lhs,
                                xw3[:, h0 + kh : h0 + kh + rows, kw : kw + DOUT],
                                start=(tap == 0),
                                stop=(tap == NTAP - 1),
                            )
                    for c, (h0, rows) in enumerate(CHUNKS):
                        copy_chunk(c, h0, rows)
                    nc.scalar.dma_start(y[CO * d0 : CO * d0 + 2 * CO, :], ot[:, :])
                else:
                    # final iteration runs chunk-major so each chunk's copy and
                    # store overlap the next chunk's matmuls -> short tail
                    for c, (h0, rows) in enumerate(CHUNKS):
                        n = rows * DOUT
                        for tap in range(NTAP):
                            kh, kw = divmod(tap, K)
                            nc.tensor.matmul(
                                pt[c][:, :n],
                                wa[:, tap * 128 : (tap + 1) * 128],
                                xw3[:, h0 + kh : h0 + kh + rows, kw : kw + DOUT],
                                start=(tap == 0),
                                stop=(tap == NTAP - 1),
                            )
                        cs = copy_chunk(c, h0, rows)
                        eng = nc.scalar if c % 2 == 0 else nc.sync
                        eng.dma_start(y[CO * d0 : CO * d0 + 2 * CO, cs], ot[:, cs])

    nc.compile()
    return nc


def _f16(a):
    return np.ascontiguousarray(np.asarray(a, dtype=np.float32).astype(np.float16))


def make_in_maps(inputs, weight, bias):
    """Host-side shard/pack: returns per-core input maps."""
    inputs = np.ascontiguousarray(np.asarray(inputs, dtype=np.float32))
    weight = np.asarray(weight, dtype=np.float32)
    bias = np.asarray(bias, dtype=np.float32)

    # combined weights: [p=(kd,ci), tap*128 + (plane, co)]
    wt5 = weight.transpose(2, 1, 3, 4, 0)  # [kd, ci, kh, kw, co]
    wcomb = np.zeros((128, NTAP, 128), np.float32)
    for kh in range(K):
        for kw in range(K):
            tap = kh * K + kw
            blk = wt5[:, :, kh, kw, :].reshape(K * CI, CO)  # [(kd ci), co]
            wcomb[0:96, tap, 0:64] = blk  # plane d0: kd 0-2 on rows 0-95
            wcomb[32:128, tap, 64:128] = blk  # plane d1: kd 0-2 on rows 32-127
    wtp = _f16(wcomb.reshape(128, NTAP * 128))

    b2 = np.ascontiguousarray(np.tile(bias, 2).reshape(2 * CO, 1))
    in_maps = []
    for c in range(B):
        xc = _f16(inputs[c].transpose(1, 0, 2, 3).reshape(DIN * CI, PLANE))
        in_maps.append({"x": xc, "wt": wtp, "b2": b2})
    return in_maps


def kernel(inputs, weight, bias, **run_kwargs):
    nc = build_program()
    in_maps = make_in_maps(inputs, weight, bias)
    res = run_bass_kernel_spmd(nc, in_maps, core_ids=list(range(B)), **run_kwargs)
    out = np.stack(
        [
            res.results[c]["y"]
            .reshape(DOUT, CO, SPP)
            .transpose(1, 0, 2)
            .reshape(CO, DOUT, DOUT, DOUT)
            for c in range(B)
        ]
    )
    return out.astype(np.float32)


# revision 22
# speedup vs baseline: 1.0326x; 1.0040x over previous
"""Conv3d (8,32,48,48,48) * (64,32,3,3,3) -> (8,64,46,46,46), valid, stride 1.

Data-parallel over batch: 1 image per NeuronCore, 8 cores. Per core the conv
is an implicit GEMM with a combined weight matrix so ONE matmul computes TWO
output planes:
  - the window tile holds 4 input planes d0..d0+3 stacked on 128 partitions
    as (kd, ci),
  - lhsT [128, 128]: cols 0-63 = plane d0's weights on rows 0-95 (kd 0-2),
    cols 64-127 = plane d0+1's weights on rows 32-127 (same values shifted
    one 32-row block down), zeros elsewhere,
  - kh, kw are free-dim offsets into the window tile, so each (kh,kw) tap is
    one accumulating K=128 x M=128 matmul; 9 taps complete a PSUM chunk,
  - tap-outer loop: each tap's weights stay loaded across all 5 row chunks
    (9 LDWEIGHTS per plane pair instead of 90),
  - PSUM -> SBUF with fused bias: chunks 0-2 on ScalarE, 3-4 on VectorE,
  - ONE input DMA and ONE output DMA per plane pair (output rows d*64+co are
    contiguous in HBM), keeping the HWDGE trigger queue nearly idle,
  - warmup matmuls on a scratch PSUM bank release the HAM clock gate while
    the first DMAs are in flight.
"""

import functools

import numpy as np

import concourse.bacc as bacc
import concourse.tile as tile
from concourse import mybir
from concourse.bass_utils import run_bass_kernel_spmd

# Problem constants (hardcoded per harness contract)
B = 8
CI = 32
DIN = 48
CO = 64
K = 3
DOUT = DIN - K + 1  # 46
SPP = DOUT * DOUT  # 2116 spatial positions per output plane
PLANE = DIN * DIN  # 2304 elements per (ci, plane)
NTAP = K * K  # 9 (kh, kw) taps per output chunk

# h'-row chunking of a 46x46 output plane into PSUM-bank-sized matmuls
CHUNKS = [(0, 10), (10, 9), (19, 9), (28, 9), (37, 9)]  # (h0, rows) -> N = rows*46

F32 = mybir.dt.float32
F16 = mybir.dt.float16

WARMUP = 12  # scratch matmuls that keep the PE busy until the first DMAs land

# row bands of the first window tile, loaded as separate tiles on alternating
# DMA paths so chunk c of the first iteration starts as soon as its band lands
BANDS = [(0, 12), (10, 12), (19, 12), (28, 12), (37, 11)]  # (row0, nrows)


@functools.lru_cache(maxsize=1)
def build_program():
    nc = bacc.Bacc("TRN2", target_bir_lowering=False, debug=False)

    x = nc.dram_tensor("x", [DIN * CI, PLANE], F16, kind="ExternalInput").ap()
    # raw weights [(kd ci), tap*64 + co]; the [128, tap*128 + (plane, co)]
    # combined matrix is built on-chip (memset + two strided copies)
    wt = nc.dram_tensor("wt", [K * CI, NTAP * CO], F16, kind="ExternalInput").ap()
    b2 = nc.dram_tensor("b2", [2 * CO, 1], F32, kind="ExternalInput").ap()
    # output rows are d*64 + co so each plane pair is one contiguous DMA
    y = nc.dram_tensor("y", [DOUT * CO, SPP], F32, kind="ExternalOutput").ap()

    with tile.TileContext(nc) as tc:
        with (
            tc.tile_pool(name="wp", bufs=1) as wpool,
            tc.tile_pool(name="xp", bufs=3) as xpool,
            tc.tile_pool(name="op", bufs=3) as opool,
            tc.tile_pool(name="p0", bufs=2, space="PSUM") as p0pool,
            tc.tile_pool(name="pa", bufs=1, space="PSUM") as papool,
            tc.tile_pool(name="ps", bufs=1, space="PSUM") as pspool,
        ):
            # Scratch PSUM bank: warmup target + "wait absorber" dummy matmuls
            # (absorbs DMA-completion waits so real matmuls only wait on their
            # PSUM slot).
            scr = pspool.tile([128, 512], F32)

            # The first loads are spread over three DMA paths so everything
            # lands early: raw weights + bias on the ACT HWDGE ring, first-
            # iteration bands 0/2/4 on the SP ring, bands 1/3 via SWDGE.
            # Warmup matmuls hold the PE busy meanwhile (HAM flips to full
            # clock ~3.4us in, so the real stream runs warm almost at once).
            wa = wpool.tile([128, NTAP * 128], F16)
            nc.gpsimd.memset(wa[:, :], 0.0)
            wraw = wpool.tile([K * CI, NTAP * CO], F16)
            nc.scalar.dma_start(wraw[:, :], wt)
            bias_t = wpool.tile([2 * CO, 1], F32)
            nc.scalar.dma_start(bias_t[:, :], b2)

            bt = []
            for c, (r0, nr) in enumerate(BANDS):
                btile = wpool.tile([128, nr * DIN], F16, name=f"band{c}")
                eng = nc.sync if c % 2 == 0 else nc.gpsimd
                eng.dma_start(btile[:, :], x[0:128, r0 * DIN : (r0 + nr) * DIN])
                bt.append(btile)

            wu = wpool.tile([128, 256], F16)
            nc.gpsimd.memset(wu[:, :], 0.0)
            for _ in range(WARMUP):
                nc.tensor.matmul(
                    scr[:, :256], wu[:, :128], wu[:, :256], start=True, stop=True
                )

            # build the combined weight matrix: plane-d0 block on rows 0-95,
            # plane-d1 block on rows 32-127, one 32-row block down
            wa3a = wa[0:96, :].rearrange("p (t c) -> p t c", c=128)
            wr3 = wraw[:, :].rearrange("p (t c) -> p t c", c=CO)
            nc.vector.tensor_scalar_add(wa3a[:, :, 0:CO], wr3, 0.0)
            # engine APs starting at partition 32 may span at most 32
            # partitions, so the shifted block is copied in 32-row pieces
            for g in range(3):
                wa3b = wa[32 * (g + 1) : 32 * (g + 2), :].rearrange(
                    "p (t c) -> p t c", c=128
                )
                nc.scalar.activation(
                    wa3b[:, :, CO:128],
                    wr3[32 * g : 32 * (g + 1)],
                    mybir.ActivationFunctionType.Identity,
                )

            # absorb the weights-copy wait
            nc.tensor.matmul(scr[:, :2], wa[:, :128], wa[:, :2], start=True, stop=True)


            for t in range(DOUT // 2):
                d0 = 2 * t
                # window tile: planes d0..d0+3 stacked on partitions (kd,ci)
                if t > 0:
                    xw = xpool.tile([128, PLANE], F16, tag="xw", name=f"xw{t}")
                    nc.sync.dma_start(xw[:, :], x[CI * d0 : CI * d0 + 128, :])
                    # absorb the window-DMA wait
                    nc.tensor.matmul(
                        scr[:, :2], wa[:, :128], xw[:, :2], start=True, stop=True
                    )
                    xw3 = xw[:, :].rearrange("p (h w) -> p h w", w=DIN)

                pt = [
                    (p0pool if c < 2 else papool).tile(
                        [128, 512], F32, tag=f"pc{c}", name=f"pc{c}_{t}"
                    )
                    for c in range(5)
                ]
                ot = opool.tile([128, SPP], F32)

                last = t == DOUT // 2 - 1

                def copy_chunk(c, h0, rows):
                    n = rows * DOUT
                    cs = slice(h0 * DOUT, h0 * DOUT + n)
                    if c < 3:
                        nc.scalar.activation(
                            ot[:, cs],
                            pt[c][:, :n],
                            mybir.ActivationFunctionType.Identity,
                            bias=bias_t[:, :],
                        )
                    else:
                        nc.vector.tensor_scalar_add(
                            ot[:, cs], pt[c][:, :n], bias_t[:, :]
                        )
                    return cs

                if t == 0:
                    # chunk-major on the first iteration: chunk c reads band
                    # tile c, so matmuls start as soon as band 0 lands
                    for c, (h0, rows) in enumerate(CHUNKS):
                        n = rows * DOUT
                        bt3 = bt[c][:, :].rearrange("p (h w) -> p h w", w=DIN)
                        for tap in range(NTAP):
                            kh, kw = divmod(tap, K)
                            nc.tensor.matmul(
                                pt[c][:, :n],
                                wa[:, tap * 128 : (tap + 1) * 128],
                                bt3[:, kh : kh + rows, kw : kw + DOUT],
                                start=(tap == 0),
                                stop=(tap == NTAP - 1),
                            )
                        copy_chunk(c, h0, rows)
                    nc.scalar.dma_start(y[CO * d0 : CO * d0 + 2 * CO, :], ot[:, :])
                elif not last:
                    for tap in range(NTAP):
                        kh, kw = divmod(tap, K)
                        lhs = wa[:, tap * 128 : (tap + 1) * 128]
                        for c, (h0, rows) in enumerate(CHUNKS):
                            n = rows * DOUT
                            nc.tensor.matmul(
                                pt[c][:, :n],
                                lhs,
                                xw3[:, h0 + kh : h0 + kh + rows, kw : kw + DOUT],
                                start=(tap == 0),
                                stop=(tap == NTAP - 1),
                            )
                    for c, (h0, rows) in enumerate(CHUNKS):
                        copy_chunk(c, h0, rows)
                    nc.scalar.dma_start(y[CO * d0 : CO * d0 + 2 * CO, :], ot[:, :])
                else:
                    # final iteration runs chunk-major so each chunk's copy and
                    # store overlap the next chunk's matmuls -> short tail
                    for c, (h0, rows) in enumerate(CHUNKS):
                        n = rows * DOUT
                        for tap in range(NTAP):
                            kh, kw = divmod(tap, K)
                            nc.tensor.matmul(
                                pt[c][:, :n],
                                wa[:, tap * 128 : (tap + 1) * 128],
                                xw3[:, h0 + kh : h0 + kh + rows, kw : kw + DOUT],
                                start=(tap == 0),
                                stop=(tap == NTAP - 1),
                            )
                        cs = copy_chunk(c, h0, rows)
                        eng = nc.scalar if c % 2 == 0 else nc.sync
                        eng.dma_start(y[CO * d0 : CO * d0 + 2 * CO, cs], ot[:, cs])

    nc.compile()
    return nc


def _f16(a):
    return np.ascontiguousarray(np.asarray(a, dtype=np.float32).astype(np.float16))


def make_in_maps(inputs, weight, bias):
    """Host-side shard/pack: returns per-core input maps."""
    inputs = np.ascontiguousarray(np.asarray(inputs, dtype=np.float32))
    weight = np.asarray(weight, dtype=np.float32)
    bias = np.asarray(bias, dtype=np.float32)

    # raw weights [p=(kd,ci), tap*64 + co]; the combined matrix is built
    # on-chip from this
    wt5 = weight.transpose(2, 1, 3, 4, 0)  # [kd, ci, kh, kw, co]
    wtp = _f16(wt5.reshape(K * CI, NTAP * CO))

    b2 = np.ascontiguousarray(np.tile(bias, 2).reshape(2 * CO, 1))
    in_maps = []
    for c in range(B):
        xc = _f16(inputs[c].transpose(1, 0, 2, 3).reshape(DIN * CI, PLANE))
        in_maps.append({"x": xc, "wt": wtp, "b2": b2})
    return in_maps


def kernel(inputs, weight, bias, **run_kwargs):
    nc = build_program()
    in_maps = make_in_maps(inputs, weight, bias)
    res = run_bass_kernel_spmd(nc, in_maps, core_ids=list(range(B)), **run_kwargs)
    out = np.stack(
        [
            res.results[c]["y"]
            .reshape(DOUT, CO, SPP)
            .transpose(1, 0, 2)
            .reshape(CO, DOUT, DOUT, DOUT)
            for c in range(B)
        ]
    )
    return out.astype(np.float32)
